# revision 40
# baseline (speedup 1.0000x reference)
"""Trainium2 Bass kernel for BasicAttention (depthwise-separable QKV conv + MHA).

Reference computation (fp32):
    x [4, 256, 64, 64] --depthwise 3x3 (pad 1)--> dw
    qkv = w_pw @ dw  (1x1 pointwise, 256 -> 768)
    4 heads x 64 dim attention over S = 64*64 = 4096 tokens per batch
    out [4, 256, 64, 64]

Sharding: 8 cores, core i handles batch b = i//2 and head-pair (0,1) or (2,3).
Each core computes the depthwise conv for its full batch (256 channels), the
pointwise projection only for its two heads' q/k/v rows, then attention.

Device-side layout tricks:
  * depthwise conv = 9 accumulating PE matmuls with diagonal weight matrices
    over shifted views of a zero-padded x held in SBUF
  * pointwise matmuls emit q^T/k^T packed as [qA^T;qB^T] (64+64 partitions) so
    the dots matmuls of the two heads land on disjoint PE row groups and run
    concurrently
  * attention is computed transposed: dots^T[t,s] = k^T(stationary) x q^T, so
    softmax exp (ScalarE) writes P^T straight into the stationary-operand
    layout that the attn@v matmul needs -- no big transposes
  * row sums of P come free as a 65th ones-column appended to v
  * logits are provably tiny (|logit| < ~0.5) so softmax skips max-subtraction
All matmuls bf16 with fp32 PSUM accumulation.
"""

import os
import sys

import numpy as np


def _ensure_imports():
    try:
        import concourse.bass  # noqa: F401
        return
    except ImportError:
        pass
    for p in (
        "/root/.axon_site",
        "/root/.axon_site/_ro/trn_rl_repo",
        "/root/.axon_site/_ro/pypackages",
        "/opt/trn_rl_repo",
        "/opt/pypackages",
    ):
        if os.path.isdir(p) and p not in sys.path:
            sys.path.append(p)
    import concourse.bass  # noqa: F401


B, C, H, W = 4, 256, 64, 64
S = H * W                     # 4096
HEADS, DH = 4, 64
SCALE = DH ** -0.5
NCORES = 8
PADW = W + 2                  # 66
SCHUNK = 512                  # s-columns processed per attention chunk
NCHUNK = S // SCHUNK          # 8
TBLK = 128                    # keys per t-block
NT = S // TBLK                # 32
NSB = SCHUNK // 128           # s-blocks of 128 rows per chunk

_compiled = None


def _build_program():
    import concourse.bacc as bacc
    import concourse.mybir as mybir
    import concourse.tile as tile
    from concourse.tile_rust import add_dep_helper
    from contextlib import ExitStack

    bf16 = mybir.dt.bfloat16
    f32 = mybir.dt.float32
    Exp = mybir.ActivationFunctionType.Exp

    nc = bacc.Bacc("TRN2", target_bir_lowering=False, debug=False,
                   num_devices=NCORES)

    xb = nc.dram_tensor("xb", [2, 128, H, W], bf16, kind="ExternalInput")
    zrow = nc.dram_tensor("zrow", [128, PADW], bf16, kind="ExternalInput")
    wdiag = nc.dram_tensor("wdiag", [2, 128, 9 * 128], bf16, kind="ExternalInput")
    wq = nc.dram_tensor("wq", [2, 128, 128], bf16, kind="ExternalInput")
    wk = nc.dram_tensor("wk", [2, 128, 128], bf16, kind="ExternalInput")
    wv = nc.dram_tensor("wv", [2, 128, 128], bf16, kind="ExternalInput")
    ident = nc.dram_tensor("ident", [128, 128], bf16, kind="ExternalInput")
    out = nc.dram_tensor("out", [2, S, DH], f32, kind="ExternalOutput")

    with tile.TileContext(nc) as tc:
        with (
            tc.tile_pool(name="persist", bufs=1) as pers,
            tc.tile_pool(name="psb", bufs=2) as ppool,
            tc.tile_pool(name="fin", bufs=4) as fin,
            tc.tile_pool(name="qps", bufs=2, space="PSUM") as qps,
            tc.tile_pool(name="avps", bufs=1, space="PSUM") as avps,
            tc.tile_pool(name="tpps", bufs=1, space="PSUM") as tpps,
            tc.tile_pool(name="jkps", bufs=1, space="PSUM") as jkps,
        ):
            # ---------------- persistent SBUF tiles ----------------
            qT_sb = pers.tile([128, S], bf16)     # [qA^T(64p); qB^T(64p)]
            kT_sb = pers.tile([128, S], bf16)
            vextA = pers.tile([128, NT * 65], bf16)  # per t-block: [v | 1]
            vextB = pers.tile([128, NT * 65], bf16)
            ident_sb = pers.tile([128, 128], bf16)
            ident_f32 = pers.tile([128, 128], f32)

            pre_ctx = ExitStack()
            pre = pre_ctx.enter_context(tc.tile_pool(name="pre", bufs=1))

            # ---------------- input DMA, priority order ----------------
            xpad = []
            wdiag_sb = []
            dw_sb = []
            for g in range(2):
                xp = pre.tile([128, PADW * PADW], bf16, name=f"xpad{g}")
                xp3 = xp.rearrange("p (h w) -> p h w", h=PADW)
                # stage x contiguously (fast DMA), spread into the padded
                # layout with a DVE copy; the dw tile doubles as staging
                # space (conv overwrites it afterwards)
                dw = pre.tile([128, S], bf16, name=f"dw{g}")
                nc.sync.dma_start(
                    out=dw[:], in_=xb[g].rearrange("p h w -> p (h w)"))
                wd = pre.tile([128, 9 * 128], bf16, name=f"wdiag{g}")
                nc.sync.dma_start(out=wd[:], in_=wdiag[g])
                # zero the padding border (memset overlaps the x DMA)
                nc.gpsimd.memset(xp[:], 0.0)
                nc.vector.tensor_copy(
                    xp3[:, 1:65, 1:65],
                    dw.rearrange("p (h w) -> p h w", h=64))
                xpad.append(xp3)
                wdiag_sb.append(wd)
                dw_sb.append(dw)

            wq_sb = pre.tile([128, 256], bf16)
            wk_sb = pre.tile([128, 256], bf16)
            wv_sb = pre.tile([128, 256], bf16)
            for kg in range(2):
                nc.sync.dma_start(out=wk_sb[:, kg * 128:(kg + 1) * 128], in_=wk[kg])
            for kg in range(2):
                nc.sync.dma_start(out=wq_sb[:, kg * 128:(kg + 1) * 128], in_=wq[kg])
                nc.sync.dma_start(out=wv_sb[:, kg * 128:(kg + 1) * 128], in_=wv[kg])
            nc.sync.dma_start(out=ident_sb[:], in_=ident[:])
            nc.gpsimd.memset(vextA[:], 1.0)
            nc.gpsimd.memset(vextB[:], 1.0)
            nc.scalar.copy(ident_f32[:], ident_sb[:])

            # ---------------- depthwise conv (borrows qps) ----------------
            # tap-outer over chunk-quads of 4 s-chunks so each LDWEIGHTS is
            # reused 4x; accumulators live in halves of two qp-sized tiles.
            # cq-outer so the pointwise k projection can start mid-conv.
            def conv_pair_mms(g, cp, cps, tap0, ntap):
                # taps [tap0, tap0+ntap) of a 2-s-chunk conv accumulation
                for t in range(tap0, tap0 + ntap):
                    dy, dx = t // 3, t % 3
                    for i in range(2):
                        h0 = (cp * 2 + i) * 8
                        nc.tensor.matmul(
                            cps[i],
                            lhsT=wdiag_sb[g][:, t * 128:(t + 1) * 128],
                            rhs=xpad[g][:, h0 + dy:h0 + dy + 8, dx:dx + 64],
                            start=(t == 0), stop=(t == 8),
                            skip_group_check=True,
                        )

            def conv_pair_drain(g, cp, cps):
                for i in range(2):
                    ch = cp * 2 + i
                    nc.vector.tensor_copy(
                        dw_sb[g][:, ch * 512:(ch + 1) * 512], cps[i])

            def conv(g, cq):
                cv = [qps.tile([128, 2 * 512], f32, name="qp")
                      for _ in range(2)]
                cps = [cv[i // 2][:, (i % 2) * 512:(i % 2 + 1) * 512]
                       for i in range(4)]
                for t in range(9):
                    dy, dx = t // 3, t % 3
                    for i in range(4):
                        h0 = (cq * 4 + i) * 8
                        nc.tensor.matmul(
                            cps[i],
                            lhsT=wdiag_sb[g][:, t * 128:(t + 1) * 128],
                            rhs=xpad[g][:, h0 + dy:h0 + dy + 8, dx:dx + 64],
                            start=(t == 0), stop=(t == 8),
                            skip_group_check=True,
                        )
                for i in range(4):
                    ch = cq * 4 + i
                    nc.vector.tensor_copy(
                        dw_sb[g][:, ch * 512:(ch + 1) * 512], cps[i])

            def pw(dst, wsb, chn, copy_engine):
                pps = avps.tile([128, 512], f32, tag=f"acc{chn % 2}",
                                name=f"acc{chn % 2}")
                for kg in range(2):
                    nc.tensor.matmul(
                        pps[:],
                        lhsT=wsb[:, kg * 128:(kg + 1) * 128],
                        rhs=dw_sb[kg][:, chn * 512:(chn + 1) * 512],
                        start=(kg == 0), stop=(kg == 1),
                        skip_group_check=True,
                    )
                if copy_engine == "act":
                    nc.scalar.copy(dst[:, chn * 512:(chn + 1) * 512], pps[:])
                else:
                    nc.vector.tensor_copy(dst[:, chn * 512:(chn + 1) * 512],
                                          pps[:])

            # k^T for all chunks, then q^T chunk 0: attention can start
            # while the rest of the pointwise work rides inside chunk 0
            vT_sb = pre.tile([128, S], bf16)
            conv(0, 0)
            conv(1, 0)
            for chn in range(4):
                pw(kT_sb, wk_sb, chn, "act")
            pw(qT_sb, wq_sb, 0, "act")

            def vtrans(t):
                tps = tpps.tile([128, 4 * 65], bf16, name="tps")
                nc.tensor.transpose(
                    tps[:, 0:128], vT_sb[:, t * 128:(t + 1) * 128], ident_sb[:])
                nc.vector.tensor_copy(
                    vextA[:, t * 65:t * 65 + 64], tps[:, 0:64])
                nc.vector.tensor_copy(
                    vextB[:, t * 65:t * 65 + 64], tps[:, 64:128])

            # ---------------- attention ----------------
            pbuf = {}
            cstate = {}

            def junk(n):
                # keep the PE HAM activity monitor busy through what would
                # otherwise be a sem-wait gap (idle gaps re-throttle the PE
                # clock to 1.2 GHz and it never recovers mid-kernel)
                for _ in range(n):
                    jt = jkps.tile([128, 512], f32, tag="junk_jk",
                                   name="junk_jk")
                    nc.tensor.matmul(
                        jt[:], lhsT=kT_sb[:, 0:128],
                        rhs=qT_sb[:, 0:512], start=True, stop=True,
                        skip_group_check=True)

            def av_mms(pb, t0, tn):
                # t-blocks [t0, t0+tn) of the attn@v accumulation
                for t in range(t0, t0 + tn):
                    for h in range(2):
                        vext = vextA if h == 0 else vextB
                        nc.tensor.matmul(
                            pb[h]["acc"][:],
                            lhsT=vext[:, t * 65:(t + 1) * 65],
                            rhs=pb["p"][:, (2 * t + h) * 512:
                                        (2 * t + h + 1) * 512],
                            start=(t == 0), stop=(t == NT - 1),
                            skip_group_check=True,
                        )

            outTs = {}

            def av_copyout(pb, c):
                # drain the finished accumulators to SBUF (frees the acc psum
                # banks without PE work in the chunk-boundary critical path)
                outTs[c] = []
                for h in range(2):
                    # acc[0:64, s] = unnormalized out^T, acc[64, s] = row sum
                    outT = fin.tile([65, 512], f32, tag="outT", name="outT",
                                    bufs=2)
                    nc.vector.tensor_copy(outT[:], pb[h]["acc"][:])
                    outTs[c].append(outT)

            def av_finish(c):
                for h in range(2):
                    outT = outTs[c][h]
                    tps = tpps.tile([128, 4 * 65], f32, name="tps")
                    for sb in range(NSB):
                        nc.tensor.transpose(
                            tps[:, sb * 65:(sb + 1) * 65],
                            outT[:, sb * 128:(sb + 1) * 128],
                            ident_f32[0:65, 0:65])
                    junk(1)
                    for sb in range(NSB):
                        sl = tps[:, sb * 65:(sb + 1) * 65]
                        rec = fin.tile([128, 1], f32, tag="rec", name="rec")
                        nc.vector.reciprocal(rec[:], sl[:, 64:65])
                        osb = fin.tile([128, 64], f32, tag="osb", name="osb")
                        nc.vector.tensor_scalar_mul(osb[:], sl[:, 0:64], rec[:])
                        nc.sync.dma_start(
                            out=out[h, c * SCHUNK + sb * 128:
                                    c * SCHUNK + sb * 128 + 128, :],
                            in_=osb[:])
                del outTs[c]

            for c in range(NCHUNK):
                # joint P layout: [128, (2t+h)*512 + s] so one ACT covers a
                # (t-block x both heads) quad
                newp = {
                    "p": ppool.tile([128, 2 * NT * SCHUNK], bf16, tag="P",
                                    name="P"),
                    0: {"acc": avps.tile([65, 512], f32, tag="acc0",
                                         name="acc0")},
                    1: {"acc": avps.tile([65, 512], f32, tag="acc1",
                                         name="acc1")},
                }
                for t in range(NT):
                    # interleave prev chunk's attn@v between dots blocks so
                    # the PE always has ready work while ACT drains quads
                    if c == 1 and t < 16:
                        vtrans(2 * t)
                        vtrans(2 * t + 1)
                    if c > 0:
                        av_mms(pbuf, t, 1)
                        junk(1)
                    else:
                        # chunk 0: the conv for s-chunks 4-7 rides here in the
                        # tpps+jkps psum banks (one (group, chunk-pair) per
                        # 5-iteration phase), k^T follows each pair, and the
                        # remaining pointwise q/v fills the later iterations
                        if t < 20:
                            phase, step = divmod(t, 5)
                            g, cp = [(0, 2), (1, 2), (0, 3), (1, 3)][phase]
                            if step == 0:
                                a = tpps.tile([128, 512], f32, name="tps")
                                b = jkps.tile([128, 512], f32, tag="junk_jk",
                                              name="junk_jk")
                                cstate[phase] = [a, b]
                            sl = [x[:] for x in cstate[phase]]
                            if step < 3:
                                conv_pair_mms(g, cp, sl, 3 * step, 3)
                            elif step == 3:
                                conv_pair_drain(g, cp, sl)
                            elif g == 1:
                                pw(kT_sb, wk_sb, 2 * cp, "dve")
                                pw(kT_sb, wk_sb, 2 * cp + 1, "dve")
                        elif t < 27:
                            pw(qT_sb, wq_sb, t - 19, "dve")
                        elif t < 31:
                            pw(vT_sb, wv_sb, 2 * (t - 27), "dve")
                            pw(vT_sb, wv_sb, 2 * (t - 27) + 1, "dve")
                        else:
                            junk(1)
                    if t == 2 and c >= 2:
                        av_finish(c - 2)
                    qp = qps.tile([128, 2 * 512], f32, name="qp")
                    prev = None
                    for h in range(2):
                        mm = nc.tensor.matmul(
                            qp[:, h * 512:(h + 1) * 512],
                            lhsT=kT_sb[h * 64:(h + 1) * 64,
                                       t * 128:(t + 1) * 128],
                            rhs=qT_sb[h * 64:(h + 1) * 64,
                                      c * SCHUNK:(c + 1) * SCHUNK],
                            start=True, stop=True,
                            skip_group_check=True,
                        )
                        # chain so the scheduler keeps A/B adjacent
                        # (different row-groups run overlapped)
                        if prev is not None:
                            add_dep_helper(mm.ins, prev.ins, True,
                                           "dots A/B interleave")
                        prev = mm
                    nc.scalar.activation(
                        newp["p"][:, 2 * t * 512:2 * (t + 1) * 512],
                        qp[:], Exp, scale=SCALE)
                if c == 0:
                    pre_ctx.close()
                if c > 0:
                    av_copyout(pbuf, c - 1)
                pbuf = newp
            # tail: final chunk's attn@v, with the previous chunk's finish
            # work interleaved into the matmul stream
            av_mms(pbuf, 0, 10)
            av_finish(NCHUNK - 2)
            av_mms(pbuf, 10, NT - 10)
            av_copyout(pbuf, NCHUNK - 1)
            av_finish(NCHUNK - 1)

    nc.compile()
    return nc


def _get_compiled():
    global _compiled
    if _compiled is None:
        _ensure_imports()
        _compiled = _build_program()
    return _compiled


def _prep_core_inputs(x, w_dw, w_pw, core):
    import ml_dtypes
    bf16 = ml_dtypes.bfloat16

    b = core // 2
    hA = 2 * (core % 2)
    hB = hA + 1

    xb = np.ascontiguousarray(x[b].reshape(2, 128, H, W)).astype(bf16)

    wd = np.zeros((2, 128, 9, 128), np.float32)
    taps = w_dw[:, 0].reshape(C, 9)          # [c, tap]
    for g in range(2):
        for t in range(9):
            np.fill_diagonal(wd[g, :, t, :], taps[g * 128:(g + 1) * 128, t])
    wdiag = wd.reshape(2, 128, 9 * 128).astype(bf16)

    def pack(base):
        # [256 c, 128] with cols 0:64 = head A rows, 64:128 = head B rows
        rows = np.concatenate([
            w_pw[base + hA * 64: base + hA * 64 + 64, :],
            w_pw[base + hB * 64: base + hB * 64 + 64, :],
        ], axis=0)                            # [128, 256]
        m = rows.T.reshape(2, 128, 128)       # [kg, c_part, o]
        return np.ascontiguousarray(m).astype(bf16)

    return {
        "xb": xb,
        "zrow": np.zeros((128, PADW), bf16),
        "wdiag": wdiag,
        "wq": pack(0),
        "wk": pack(C),
        "wv": pack(2 * C),
        "ident": np.eye(128, dtype=bf16),
    }


def kernel(x, w_dw, w_pw, _trace=False, _tmpdir=None):
    _ensure_imports()
    from concourse.bass_utils import run_bass_kernel_spmd

    x = np.asarray(x, dtype=np.float32)
    w_dw = np.asarray(w_dw, dtype=np.float32)
    w_pw = np.asarray(w_pw, dtype=np.float32)

    nc = _get_compiled()
    in_maps = [_prep_core_inputs(x, w_dw, w_pw, i) for i in range(NCORES)]
    res = run_bass_kernel_spmd(nc, in_maps, list(range(NCORES)),
                               trace=_trace, tmpdir=_tmpdir)

    full = np.empty((B, C, H, W), np.float32)
    for i in range(NCORES):
        b = i // 2
        oc = res.results[i]["out"]            # [2, S, DH]
        for j in range(2):
            h = 2 * (i % 2) + j
            full[b, h * 64:(h + 1) * 64] = oc[j].T.reshape(DH, H, W)
    if _trace:
        return full, res
    return full


# revision 41
# speedup vs baseline: 1.0009x; 1.0009x over previous
"""Trainium2 Bass kernel for BasicAttention (depthwise-separable QKV conv + MHA).

Reference computation (fp32):
    x [4, 256, 64, 64] --depthwise 3x3 (pad 1)--> dw
    qkv = w_pw @ dw  (1x1 pointwise, 256 -> 768)
    4 heads x 64 dim attention over S = 64*64 = 4096 tokens per batch
    out [4, 256, 64, 64]

Sharding: 8 cores, core i handles batch b = i//2 and head-pair (0,1) or (2,3).
Each core computes the depthwise conv for its full batch (256 channels), the
pointwise projection only for its two heads' q/k/v rows, then attention.

Device-side layout tricks:
  * depthwise conv = 9 accumulating PE matmuls with diagonal weight matrices
    over shifted views of a zero-padded x held in SBUF
  * pointwise matmuls emit q^T/k^T packed as [qA^T;qB^T] (64+64 partitions) so
    the dots matmuls of the two heads land on disjoint PE row groups and run
    concurrently
  * attention is computed transposed: dots^T[t,s] = k^T(stationary) x q^T, so
    softmax exp (ScalarE) writes P^T straight into the stationary-operand
    layout that the attn@v matmul needs -- no big transposes
  * row sums of P come free as a 65th ones-column appended to v
  * logits are provably tiny (|logit| < ~0.5) so softmax skips max-subtraction
All matmuls bf16 with fp32 PSUM accumulation.
"""

import os
import sys

import numpy as np


def _ensure_imports():
    try:
        import concourse.bass  # noqa: F401
        return
    except ImportError:
        pass
    for p in (
        "/root/.axon_site",
        "/root/.axon_site/_ro/trn_rl_repo",
        "/root/.axon_site/_ro/pypackages",
        "/opt/trn_rl_repo",
        "/opt/pypackages",
    ):
        if os.path.isdir(p) and p not in sys.path:
            sys.path.append(p)
    import concourse.bass  # noqa: F401


B, C, H, W = 4, 256, 64, 64
S = H * W                     # 4096
HEADS, DH = 4, 64
SCALE = DH ** -0.5
NCORES = 8
PADW = W + 2                  # 66
SCHUNK = 512                  # s-columns processed per attention chunk
NCHUNK = S // SCHUNK          # 8
TBLK = 128                    # keys per t-block
NT = S // TBLK                # 32
NSB = SCHUNK // 128           # s-blocks of 128 rows per chunk

_compiled = None


def _build_program():
    import concourse.bacc as bacc
    import concourse.mybir as mybir
    import concourse.tile as tile
    from concourse.tile_rust import add_dep_helper
    from contextlib import ExitStack

    bf16 = mybir.dt.bfloat16
    f32 = mybir.dt.float32
    Exp = mybir.ActivationFunctionType.Exp

    nc = bacc.Bacc("TRN2", target_bir_lowering=False, debug=False,
                   num_devices=NCORES)

    xb = nc.dram_tensor("xb", [2, 128, H, W], bf16, kind="ExternalInput")
    zrow = nc.dram_tensor("zrow", [128, PADW], bf16, kind="ExternalInput")
    wdiag = nc.dram_tensor("wdiag", [2, 128, 9 * 128], bf16, kind="ExternalInput")
    wq = nc.dram_tensor("wq", [2, 128, 128], bf16, kind="ExternalInput")
    wk = nc.dram_tensor("wk", [2, 128, 128], bf16, kind="ExternalInput")
    wv = nc.dram_tensor("wv", [2, 128, 128], bf16, kind="ExternalInput")
    ident = nc.dram_tensor("ident", [128, 128], bf16, kind="ExternalInput")
    out = nc.dram_tensor("out", [2, S, DH], f32, kind="ExternalOutput")

    with tile.TileContext(nc) as tc:
        with (
            tc.tile_pool(name="persist", bufs=1) as pers,
            tc.tile_pool(name="psb", bufs=2) as ppool,
            tc.tile_pool(name="fin", bufs=4) as fin,
            tc.tile_pool(name="qps", bufs=2, space="PSUM") as qps,
            tc.tile_pool(name="avps", bufs=1, space="PSUM") as avps,
            tc.tile_pool(name="tpps", bufs=1, space="PSUM") as tpps,
            tc.tile_pool(name="jkps", bufs=1, space="PSUM") as jkps,
        ):
            # ---------------- persistent SBUF tiles ----------------
            qT_sb = pers.tile([128, S], bf16)     # [qA^T(64p); qB^T(64p)]
            kT_sb = pers.tile([128, S], bf16)
            vextA = pers.tile([128, NT * 65], bf16)  # per t-block: [v | 1]
            vextB = pers.tile([128, NT * 65], bf16)
            ident_sb = pers.tile([128, 128], bf16)
            ident_f32 = pers.tile([128, 128], f32)

            pre_ctx = ExitStack()
            pre = pre_ctx.enter_context(tc.tile_pool(name="pre", bufs=1))

            # ---------------- input DMA, priority order ----------------
            xpad = []
            wdiag_sb = []
            dw_sb = []
            for g in range(2):
                xp = pre.tile([128, PADW * PADW], bf16, name=f"xpad{g}")
                xp3 = xp.rearrange("p (h w) -> p h w", h=PADW)
                # stage x contiguously (fast DMA), spread into the padded
                # layout with a DVE copy; the dw tile doubles as staging
                # space (conv overwrites it afterwards)
                dw = pre.tile([128, S], bf16, name=f"dw{g}")
                nc.sync.dma_start(
                    out=dw[:], in_=xb[g].rearrange("p h w -> p (h w)"))
                wd = pre.tile([128, 9 * 128], bf16, name=f"wdiag{g}")
                nc.sync.dma_start(out=wd[:], in_=wdiag[g])
                # zero the padding border (memset overlaps the x DMA)
                nc.gpsimd.memset(xp[:], 0.0)
                nc.vector.tensor_copy(
                    xp3[:, 1:65, 1:65],
                    dw.rearrange("p (h w) -> p h w", h=64))
                xpad.append(xp3)
                wdiag_sb.append(wd)
                dw_sb.append(dw)

            wq_sb = pre.tile([128, 256], bf16)
            wk_sb = pre.tile([128, 256], bf16)
            wv_sb = pre.tile([128, 256], bf16)
            for kg in range(2):
                nc.sync.dma_start(out=wk_sb[:, kg * 128:(kg + 1) * 128], in_=wk[kg])
            for kg in range(2):
                nc.sync.dma_start(out=wq_sb[:, kg * 128:(kg + 1) * 128], in_=wq[kg])
                nc.sync.dma_start(out=wv_sb[:, kg * 128:(kg + 1) * 128], in_=wv[kg])
            nc.sync.dma_start(out=ident_sb[:], in_=ident[:])
            nc.gpsimd.memset(vextA[:], 1.0)
            nc.gpsimd.memset(vextB[:], 1.0)
            nc.scalar.copy(ident_f32[:], ident_sb[:])

            # ---------------- depthwise conv (borrows qps) ----------------
            # tap-outer over chunk-quads of 4 s-chunks so each LDWEIGHTS is
            # reused 4x; accumulators live in halves of two qp-sized tiles.
            # cq-outer so the pointwise k projection can start mid-conv.
            def conv_pair_mms(g, cp, cps, tap0, ntap):
                # taps [tap0, tap0+ntap) of a 2-s-chunk conv accumulation
                for t in range(tap0, tap0 + ntap):
                    dy, dx = t // 3, t % 3
                    for i in range(2):
                        h0 = (cp * 2 + i) * 8
                        nc.tensor.matmul(
                            cps[i],
                            lhsT=wdiag_sb[g][:, t * 128:(t + 1) * 128],
                            rhs=xpad[g][:, h0 + dy:h0 + dy + 8, dx:dx + 64],
                            start=(t == 0), stop=(t == 8),
                            skip_group_check=True,
                        )

            def conv_pair_drain(g, cp, cps):
                for i in range(2):
                    ch = cp * 2 + i
                    nc.vector.tensor_copy(
                        dw_sb[g][:, ch * 512:(ch + 1) * 512], cps[i])

            def conv(g, cq):
                cv = [qps.tile([128, 2 * 512], f32, name="qp")
                      for _ in range(2)]
                cps = [cv[i // 2][:, (i % 2) * 512:(i % 2 + 1) * 512]
                       for i in range(4)]
                for t in range(9):
                    dy, dx = t // 3, t % 3
                    for i in range(4):
                        h0 = (cq * 4 + i) * 8
                        nc.tensor.matmul(
                            cps[i],
                            lhsT=wdiag_sb[g][:, t * 128:(t + 1) * 128],
                            rhs=xpad[g][:, h0 + dy:h0 + dy + 8, dx:dx + 64],
                            start=(t == 0), stop=(t == 8),
                            skip_group_check=True,
                        )
                for i in range(4):
                    ch = cq * 4 + i
                    nc.vector.tensor_copy(
                        dw_sb[g][:, ch * 512:(ch + 1) * 512], cps[i])

            def pw(dst, wsb, chn, copy_engine):
                pps = avps.tile([128, 512], f32, tag=f"acc{chn % 2}",
                                name=f"acc{chn % 2}")
                for kg in range(2):
                    nc.tensor.matmul(
                        pps[:],
                        lhsT=wsb[:, kg * 128:(kg + 1) * 128],
                        rhs=dw_sb[kg][:, chn * 512:(chn + 1) * 512],
                        start=(kg == 0), stop=(kg == 1),
                        skip_group_check=True,
                    )
                if copy_engine == "act":
                    nc.scalar.copy(dst[:, chn * 512:(chn + 1) * 512], pps[:])
                else:
                    nc.vector.tensor_copy(dst[:, chn * 512:(chn + 1) * 512],
                                          pps[:])

            # k^T for all chunks, then q^T chunk 0: attention can start
            # while the rest of the pointwise work rides inside chunk 0
            vT_sb = pre.tile([128, S], bf16)
            conv(0, 0)
            conv(1, 0)
            for chn in range(4):
                pw(kT_sb, wk_sb, chn, "act")
            pw(qT_sb, wq_sb, 0, "act")

            def vtrans(t):
                tps = tpps.tile([128, 4 * 65], bf16, name="tps")
                nc.tensor.transpose(
                    tps[:, 0:128], vT_sb[:, t * 128:(t + 1) * 128], ident_sb[:])
                nc.vector.tensor_copy(
                    vextA[:, t * 65:t * 65 + 64], tps[:, 0:64])
                nc.vector.tensor_copy(
                    vextB[:, t * 65:t * 65 + 64], tps[:, 64:128])

            # ---------------- attention ----------------
            pbuf = {}
            cstate = {}

            def junk(n):
                # keep the PE HAM activity monitor busy through what would
                # otherwise be a sem-wait gap (idle gaps re-throttle the PE
                # clock to 1.2 GHz and it never recovers mid-kernel)
                for _ in range(n):
                    jt = jkps.tile([128, 512], f32, tag="junk_jk",
                                   name="junk_jk")
                    nc.tensor.matmul(
                        jt[:], lhsT=kT_sb[:, 0:128],
                        rhs=qT_sb[:, 0:512], start=True, stop=True,
                        skip_group_check=True)

            def av_mms(pb, t0, tn):
                # t-blocks [t0, t0+tn) of the attn@v accumulation
                for t in range(t0, t0 + tn):
                    for h in range(2):
                        vext = vextA if h == 0 else vextB
                        nc.tensor.matmul(
                            pb[h]["acc"][:],
                            lhsT=vext[:, t * 65:(t + 1) * 65],
                            rhs=pb["p"][:, (2 * t + h) * 512:
                                        (2 * t + h + 1) * 512],
                            start=(t == 0), stop=(t == NT - 1),
                            skip_group_check=True,
                        )

            outTs = {}

            def av_copyout(pb, c):
                # drain the finished accumulators to SBUF (frees the acc psum
                # banks without PE work in the chunk-boundary critical path)
                outTs[c] = []
                for h in range(2):
                    # acc[0:64, s] = unnormalized out^T, acc[64, s] = row sum
                    outT = fin.tile([65, 512], f32, tag="outT", name="outT",
                                    bufs=2)
                    nc.vector.tensor_copy(outT[:], pb[h]["acc"][:])
                    outTs[c].append(outT)

            def av_finish(c):
                for h in range(2):
                    outT = outTs[c][h]
                    tps = tpps.tile([128, 4 * 65], f32, name="tps")
                    for sb in range(NSB):
                        nc.tensor.transpose(
                            tps[:, sb * 65:(sb + 1) * 65],
                            outT[:, sb * 128:(sb + 1) * 128],
                            ident_f32[0:65, 0:65])
                    junk(1)
                    for sb in range(NSB):
                        sl = tps[:, sb * 65:(sb + 1) * 65]
                        rec = fin.tile([128, 1], f32, tag="rec", name="rec")
                        nc.vector.reciprocal(rec[:], sl[:, 64:65])
                        osb = fin.tile([128, 64], f32, tag="osb", name="osb")
                        nc.vector.tensor_scalar_mul(osb[:], sl[:, 0:64], rec[:])
                        nc.sync.dma_start(
                            out=out[h, c * SCHUNK + sb * 128:
                                    c * SCHUNK + sb * 128 + 128, :],
                            in_=osb[:])
                del outTs[c]

            for c in range(NCHUNK):
                # joint P layout: [128, (2t+h)*512 + s] so one ACT covers a
                # (t-block x both heads) quad
                newp = {
                    "p": ppool.tile([128, 2 * NT * SCHUNK], bf16, tag="P",
                                    name="P"),
                    0: {"acc": avps.tile([65, 512], f32, tag="acc0",
                                         name="acc0")},
                    1: {"acc": avps.tile([65, 512], f32, tag="acc1",
                                         name="acc1")},
                }
                for t in range(NT):
                    # interleave prev chunk's attn@v between dots blocks so
                    # the PE always has ready work while ACT drains quads
                    if c == 1 and t < 16:
                        vtrans(2 * t)
                        vtrans(2 * t + 1)
                    if c > 0:
                        # from chunk 2 on, run the attn@v interleave 2 blocks
                        # ahead so the previous P buffer is released before
                        # the chunk boundary (unblocks next chunk's first
                        # exp); chunk 1 stays aligned with the v-transpose
                        # production it depends on
                        if c == 1:
                            av_mms(pbuf, t, 1)
                        elif t == 0:
                            av_mms(pbuf, 0, 3)
                        elif t <= 29:
                            av_mms(pbuf, t + 2, 1)
                        junk(1)
                    else:
                        # chunk 0: the conv for s-chunks 4-7 rides here in the
                        # tpps+jkps psum banks (one (group, chunk-pair) per
                        # 5-iteration phase), k^T follows each pair, and the
                        # remaining pointwise q/v fills the later iterations
                        if t < 20:
                            phase, step = divmod(t, 5)
                            g, cp = [(0, 2), (1, 2), (0, 3), (1, 3)][phase]
                            if step == 0:
                                a = tpps.tile([128, 512], f32, name="tps")
                                b = jkps.tile([128, 512], f32, tag="junk_jk",
                                              name="junk_jk")
                                cstate[phase] = [a, b]
                            sl = [x[:] for x in cstate[phase]]
                            if step < 3:
                                conv_pair_mms(g, cp, sl, 3 * step, 3)
                            elif step == 3:
                                conv_pair_drain(g, cp, sl)
                            elif g == 1:
                                pw(kT_sb, wk_sb, 2 * cp, "dve")
                                pw(kT_sb, wk_sb, 2 * cp + 1, "dve")
                        elif t < 27:
                            pw(qT_sb, wq_sb, t - 19, "dve")
                        elif t < 31:
                            pw(vT_sb, wv_sb, 2 * (t - 27), "dve")
                            pw(vT_sb, wv_sb, 2 * (t - 27) + 1, "dve")
                        else:
                            junk(1)
                    if t == 2 and c >= 2:
                        av_finish(c - 2)
                    qp = qps.tile([128, 2 * 512], f32, name="qp")
                    prev = None
                    for h in range(2):
                        mm = nc.tensor.matmul(
                            qp[:, h * 512:(h + 1) * 512],
                            lhsT=kT_sb[h * 64:(h + 1) * 64,
                                       t * 128:(t + 1) * 128],
                            rhs=qT_sb[h * 64:(h + 1) * 64,
                                      c * SCHUNK:(c + 1) * SCHUNK],
                            start=True, stop=True,
                            skip_group_check=True,
                        )
                        # chain so the scheduler keeps A/B adjacent
                        # (different row-groups run overlapped)
                        if prev is not None:
                            add_dep_helper(mm.ins, prev.ins, True,
                                           "dots A/B interleave")
                        prev = mm
                    nc.scalar.activation(
                        newp["p"][:, 2 * t * 512:2 * (t + 1) * 512],
                        qp[:], Exp, scale=SCALE)
                if c == 0:
                    pre_ctx.close()
                if c > 0:
                    av_copyout(pbuf, c - 1)
                pbuf = newp
            # tail: final chunk's attn@v, with the previous chunk's finish
            # work interleaved into the matmul stream
            av_mms(pbuf, 0, 10)
            av_finish(NCHUNK - 2)
            av_mms(pbuf, 10, NT - 10)
            av_copyout(pbuf, NCHUNK - 1)
            av_finish(NCHUNK - 1)

    nc.compile()
    return nc


def _get_compiled():
    global _compiled
    if _compiled is None:
        _ensure_imports()
        _compiled = _build_program()
    return _compiled


def _prep_core_inputs(x, w_dw, w_pw, core):
    import ml_dtypes
    bf16 = ml_dtypes.bfloat16

    b = core // 2
    hA = 2 * (core % 2)
    hB = hA + 1

    xb = np.ascontiguousarray(x[b].reshape(2, 128, H, W)).astype(bf16)

    wd = np.zeros((2, 128, 9, 128), np.float32)
    taps = w_dw[:, 0].reshape(C, 9)          # [c, tap]
    for g in range(2):
        for t in range(9):
            np.fill_diagonal(wd[g, :, t, :], taps[g * 128:(g + 1) * 128, t])
    wdiag = wd.reshape(2, 128, 9 * 128).astype(bf16)

    def pack(base):
        # [256 c, 128] with cols 0:64 = head A rows, 64:128 = head B rows
        rows = np.concatenate([
            w_pw[base + hA * 64: base + hA * 64 + 64, :],
            w_pw[base + hB * 64: base + hB * 64 + 64, :],
        ], axis=0)                            # [128, 256]
        m = rows.T.reshape(2, 128, 128)       # [kg, c_part, o]
        return np.ascontiguousarray(m).astype(bf16)

    return {
        "xb": xb,
        "zrow": np.zeros((128, PADW), bf16),
        "wdiag": wdiag,
        "wq": pack(0),
        "wk": pack(C),
        "wv": pack(2 * C),
        "ident": np.eye(128, dtype=bf16),
    }


def kernel(x, w_dw, w_pw, _trace=False, _tmpdir=None):
    _ensure_imports()
    from concourse.bass_utils import run_bass_kernel_spmd

    x = np.asarray(x, dtype=np.float32)
    w_dw = np.asarray(w_dw, dtype=np.float32)
    w_pw = np.asarray(w_pw, dtype=np.float32)

    nc = _get_compiled()
    in_maps = [_prep_core_inputs(x, w_dw, w_pw, i) for i in range(NCORES)]
    res = run_bass_kernel_spmd(nc, in_maps, list(range(NCORES)),
                               trace=_trace, tmpdir=_tmpdir)

    full = np.empty((B, C, H, W), np.float32)
    for i in range(NCORES):
        b = i // 2
        oc = res.results[i]["out"]            # [2, S, DH]
        for j in range(2):
            h = 2 * (i % 2) + j
            full[b, h * 64:(h + 1) * 64] = oc[j].T.reshape(DH, H, W)
    if _trace:
        return full, res
    return full
